# revision 20
# baseline (speedup 1.0000x reference)
"""Trainium2 Bass kernel for nn_ConvQuantizationWrapper.

The reference bit-slices an 8-bit quantized 3x3 conv into 32 (2-bit act x
1-bit weight) conv passes and recombines them with powers of two. That
decomposition exactly reconstructs

    out = conv2d(A, Wq) / (sa*sw) + bias
    A   = clip(round(x*sa - zp), 0, 255) + zp        (integers in [-128,127])
    Wq  = wrap_int8(round(w*sw))                     (integers in [-128,127])

in exact integer arithmetic (all partial sums < 2^24, so bf16-input
matmuls with fp32 PSUM accumulation are exact). The kernel runs one
quantized conv, data-parallel over batch (8 images per NeuronCore).

Per image pair (2 images = 128 partitions of staging):
  - ACT: u = x*sa + (MAGIC - zp)  (fused affine; MAGIC forces RNE-to-int)
  - DVE: A-slot writes  A = u - (MAGIC - zp) -> bf16 padded [58,58] frames
         in T1[0:64], plus one row-shifted copy B = A shift (1,0) into
         T1[64:128] (K=128 tap pairing needs taps (kh),(kh+1) at the same
         within-partition offset).
  - PE: 3x3 conv as 6 matmul groups per 8-row output chunk, all K=128
        (64ch x 2 kernel rows) x M=64 x N=448:
           g0..g2: taps {(0,q),(1,q)}  from T1 at (R,   q)
           g3..g5: tap  {(2,q)} via B-half at (R+1, q), A-half weights = 0
        Per chunk/bank: img0's full 6-matmul accumulation group completes,
        then img1's (never two groups open in one bank; uniform rows).
  - ACT epilogue: y = psum * (1/(sa*sw)) + bias
"""

import numpy as np
import ml_dtypes

import concourse.bacc as bacc
import concourse.mybir as mybir
import concourse.tile as tile
from concourse import bass_utils

N_CORES = 8
IMGS = 8          # images per core (batch 64 / 8 cores)
C = 64
H = W = 56
HP = 58           # padded spatial
NPIX = H * W      # 3136
CHUNK_ROWS = 8
CHUNK = CHUNK_ROWS * W   # 448 output pixels per PSUM bank
NCHUNKS = H // CHUNK_ROWS
MAGIC = 12582912.0       # 1.5 * 2**23: float32 round-to-nearest-integer trick
NGROUPS = 6

_nc_cache = {}


def _build(sa: float, neg_zp: float, recip: float):
    """Build + compile the per-core Bass kernel (cached per scalar config)."""
    key = (sa, neg_zp, recip)
    if key in _nc_cache:
        return _nc_cache[key]

    A = mybir.AluOpType
    F = mybir.ActivationFunctionType
    nc = bacc.Bacc("TRN2", target_bir_lowering=False, debug=False)
    x_d = nc.dram_tensor("x", [IMGS, C, H, W], mybir.dt.float32,
                         kind="ExternalInput").ap()
    w_d = nc.dram_tensor("wt", [128, NGROUPS * 64], mybir.dt.bfloat16,
                         kind="ExternalInput").ap()
    b_d = nc.dram_tensor("biasd", [128, 1], mybir.dt.float32,
                         kind="ExternalInput").ap()
    y_d = nc.dram_tensor("y", [IMGS, C, H, W], mybir.dt.float32,
                         kind="ExternalOutput").ap()

    # u = x*sa + (MAGIC - zp); A-slot = u - (MAGIC - zp)
    u_bias = MAGIC + neg_zp          # MAGIC - zp
    a_bias = -(MAGIC + neg_zp)

    with tile.TileContext(nc) as tc:
        with (
            tc.tile_pool(name="const", bufs=1) as cpool,
            tc.tile_pool(name="xbuf", bufs=1) as xpool,
            tc.tile_pool(name="xin", bufs=4) as xinpool,
            tc.tile_pool(name="work", bufs=2) as wpool,
            tc.tile_pool(name="psum", bufs=8, space="PSUM") as ppool,
        ):
            w_sb = cpool.tile([128, NGROUPS * 64], mybir.dt.bfloat16,
                              name="w_sb")
            nc.sync.dma_start(out=w_sb, in_=w_d)
            b_sb = cpool.tile([128, 1], mybir.dt.float32, name="b_sb")
            nc.sync.dma_start(out=b_sb, in_=b_d)
            # dummy activation: forces the ACT table load off the
            # critical path (it otherwise fires right before the first
            # real epilogue, after the first matmul group completes)
            warm = cpool.tile([128, 1], mybir.dt.float32, name="warm")
            nc.scalar.activation(out=warm, in_=b_sb, func=F.Identity,
                                 bias=b_sb, scale=1.0)

            # Persistent per-pair layout tiles [slot-half, img, 58,58]
            # (4 sets = one per pair: quant never WAR-waits on matmuls).
            # Only the A-half borders need zeroing (reads of the pad
            # frame); the B-half is fully covered by the shifted copy.
            # Memsets run on the idle GpSimd engine (partition-local) so
            # the DVE quant chain starts the moment the first DMA lands.
            Tbufs = []
            for j in range(IMGS // 2):
                T1 = xpool.tile([128, 2, HP, HP], mybir.dt.bfloat16,
                                name=f"T1_{j}", tag=f"T1_{j}")
                nc.gpsimd.memset(T1[0:64, :, 0:1, :], 0.0)    # top pad row
                nc.gpsimd.memset(T1[0:64, :, 57:58, :], 0.0)  # bottom pad
                nc.gpsimd.memset(T1[0:64, :, :, 0:1], 0.0)    # left pad col
                nc.gpsimd.memset(T1[0:64, :, :, 57:58], 0.0)  # right pad
                Tbufs.append(T1)

            # PE warm-up: dummy matmuls bridge the gap between the weight
            # DMA landing and the first real chunk, so the HAM clock gate
            # warms before/while the real stream starts.
            warm_ps = ppool.tile([128, CHUNK], mybir.dt.float32,
                                 name="warm_ps", tag="ps")
            for _ in range(8):
                nc.tensor.matmul(warm_ps[0:64, 0:384],
                                 w_sb[0:128, 0:64],
                                 w_sb[0:128, 0:384],
                                 start=True, stop=True)

            def quant_rows(T1, u, h0, h1):
                """A-slot interiors for img rows [h0,h1) + the B rows they
                enable (B row h = A row h+1 = img row h)."""
                for i in range(2):
                    nc.vector.tensor_scalar(
                        T1[0:64, i:i + 1, 1 + h0:1 + h1, 1:57],
                        u[64 * i:64 * i + 64, h0 * W:h1 * W].rearrange(
                            "p (h w) -> p h w", h=h1 - h0),
                        a_bias, None, op0=A.add)
                # B rows h0..h1-1 <- A rows h0+1..h1 (all cols, incl pad);
                # at h1 == H also copy B row 56 <- A pad row 57
                b1 = h1 if h1 < H else H + 1
                nc.vector.tensor_copy(
                    T1[64:128, :, h0:b1, :],
                    T1[0:64, :, h0 + 1:b1 + 1, :])

            def conv_chunks(T1, ystage, ch_lo, ch_hi):
                for ch in range(ch_lo, ch_hi):
                    R = CHUNK_ROWS * ch
                    ps = ppool.tile([128, CHUNK], mybir.dt.float32,
                                    name="ps", tag="ps")
                    # per chunk: img0's full 6-group accumulation, then
                    # img1's (one open group per bank at a time)
                    for half, i in ((0, 0), (64, 1)):
                        for g in range(NGROUPS):
                            r0 = R if g < 3 else R + 1
                            c0 = g if g < 3 else g - 3
                            mov = T1[0:128, i:i + 1, r0:r0 + CHUNK_ROWS,
                                     c0:c0 + 56]
                            lhsT = w_sb[0:128, g * 64:(g + 1) * 64]
                            nc.tensor.matmul(ps[half:half + 64], lhsT, mov,
                                             start=(g == 0),
                                             stop=(g == NGROUPS - 1))
                    # epilogue: y = psum * recip + bias (per-partition)
                    nc.scalar.activation(
                        out=ystage[:, ch * CHUNK:(ch + 1) * CHUNK],
                        in_=ps,
                        func=F.Identity,
                        bias=b_sb, scale=recip)

            NPAIR = IMGS // 2
            xfs, us = {}, {}

            def psplits(pair):
                # first pair goes quarter-wise to start matmuls ASAP
                bounds = ((0, 16, 16, 32, 32, 48, 48, 56) if pair == 0
                          else (0, 32, 32, 56))
                return [(bounds[k], bounds[k + 1])
                        for k in range(0, len(bounds), 2)]

            def load_pair(pair):
                xf = xinpool.tile([128, NPIX], mybir.dt.float32,
                                  name="xf", tag="xf")
                x_ap = x_d[2 * pair:2 * pair + 2].rearrange(
                    "i c h w -> (i c) (h w)")
                xfs[pair] = xf
                for h0, h1 in psplits(pair):
                    nc.sync.dma_start(out=xf[:, h0 * W:h1 * W],
                                      in_=x_ap[:, h0 * W:h1 * W])

            def quant_pair(pair):
                """DVE quant chain, emitted ahead of the previous pair's
                conv block so the DVE FIFO never traps it behind later
                work."""
                T1 = Tbufs[pair]
                xf = xfs[pair]
                u = wpool.tile([128, NPIX], mybir.dt.float32,
                               name="u", tag="u")
                us[pair] = u
                for h0, h1 in psplits(pair):
                    # u = x*sa + (MAGIC - zp): DVE 2x mode beats ACT 1x,
                    # and keeps the whole quant chain in one engine FIFO
                    nc.vector.tensor_scalar(
                        u[:, h0 * W:h1 * W], xf[:, h0 * W:h1 * W],
                        sa, u_bias, op0=A.mult, op1=A.add)
                    quant_rows(T1, u, h0, h1)

            def conv_pair(pair):
                T1 = Tbufs[pair]
                y_ap = y_d[2 * pair:2 * pair + 2].rearrange(
                    "i c h w -> (i c) (h w)")
                ystage = wpool.tile([128, NPIX], mybir.dt.float32,
                                    name="ystage", tag="ystage")
                conv_chunks(T1, ystage, 0, 3)
                nc.sync.dma_start(out=y_ap[:, :3 * CHUNK],
                                  in_=ystage[:, :3 * CHUNK])
                if pair < NPAIR - 1:
                    conv_chunks(T1, ystage, 3, NCHUNKS)
                    nc.sync.dma_start(out=y_ap[:, 3 * CHUNK:],
                                      in_=ystage[:, 3 * CHUNK:])
                else:
                    # last pair: taper the output DMA so the final
                    # transfer after the last matmul is small
                    conv_chunks(T1, ystage, 3, 6)
                    nc.sync.dma_start(out=y_ap[:, 3 * CHUNK:6 * CHUNK],
                                      in_=ystage[:, 3 * CHUNK:6 * CHUNK])
                    conv_chunks(T1, ystage, 6, NCHUNKS)
                    nc.sync.dma_start(out=y_ap[:, 6 * CHUNK:],
                                      in_=ystage[:, 6 * CHUNK:])

            # software-pipelined emission: all input DMAs prefetch up
            # front; quant of pair p+1 enqueued before the conv/epilogue
            # block of pair p
            for pair in range(NPAIR):
                load_pair(pair)
            quant_pair(0)
            quant_pair(1)
            for pair in range(NPAIR):
                if pair + 2 < NPAIR:
                    quant_pair(pair + 2)
                conv_pair(pair)

    nc.compile()
    _nc_cache[key] = nc
    return nc


def _prep(x, weight, bias, scale_a, scale_w, zero_point):
    x = np.ascontiguousarray(np.asarray(x, dtype=np.float32))
    weight = np.asarray(weight, dtype=np.float32)
    bias = np.asarray(bias, dtype=np.float32)
    sa = float(np.asarray(scale_a).reshape(-1)[0])
    sw = float(np.asarray(scale_w).reshape(-1)[0])
    zp = float(np.asarray(zero_point).reshape(-1)[0])

    # activation-clip guard: reference clips round(x*sa - zp) to [0, 255].
    # For in-range data the clip is a no-op; if any value could clip,
    # pre-clamp x on the host (preserves the reference's semantics).
    amax = float(np.abs(x).max())
    if not (amax * abs(sa) < abs(zp if zp != 0 else 0) + 126.99 and
            -0.49 < -zp and sa * amax - zp < 255.49):
        f32 = np.float32
        lo = (f32(-0.49) + f32(zp)) / f32(sa)
        hi = (f32(255.49) + f32(zp)) / f32(sa)
        x = np.clip(x, lo, hi).astype(np.float32)

    # weight quantization, matching jnp.round(weight * sw) in f32 + the
    # implicit 8-bit two's-complement wrap of the bit decomposition
    qw = np.round(weight * np.float32(sw))
    qwi = qw.astype(np.int64)
    qw_eff = ((qwi + 128) % 256) - 128
    delta = qwi - qw_eff          # nonzero only if |qw| > 127 (never for
    # randn*20 weights); handled via a host-side correction plane below.

    wt = qw_eff.astype(np.float32)      # [o, i, 3, 3]

    def tap(kh, kw):
        return np.ascontiguousarray(wt[:, :, kh, kw].T)   # [in, out]

    wg = np.zeros((128, NGROUPS * 64), np.float32)
    for g in range(3):                   # {(0,g),(1,g)} pairs
        wg[0:64, g * 64:(g + 1) * 64] = tap(0, g)
        wg[64:128, g * 64:(g + 1) * 64] = tap(1, g)
    for g in range(3):                   # singles {(2,g)} via B-half
        wg[64:128, (3 + g) * 64:(4 + g) * 64] = tap(2, g)
    wg_bf = np.ascontiguousarray(wg.astype(ml_dtypes.bfloat16))

    bias_dup = np.ascontiguousarray(
        np.concatenate([bias, bias])[:, None].astype(np.float32))

    sprod = np.float32(sw) * np.float32(sa)
    recip = float(np.float32(1.0) / sprod)

    corr = None
    if np.any(delta != 0):
        # reference's zero-point term uses the unwrapped qw:
        # out_ref - out_dev = zp * conv2d(ones, delta) * recip
        dsum = delta.sum(axis=1).astype(np.float64)  # [o, 3, 3]
        plane = np.zeros((C, H, W), np.float64)
        for kh in range(3):
            for kw in range(3):
                h0, h1 = max(0, 1 - kh), min(H, H + 1 - kh)
                w0, w1 = max(0, 1 - kw), min(W, W + 1 - kw)
                plane[:, h0:h1, w0:w1] += dsum[:, kh, kw][:, None, None]
        corr = (zp * plane * float(recip)).astype(np.float32)

    return x, wg_bf, bias_dup, sa, zp, recip, corr


def _run(x, weight, bias, scale_a, scale_w, zero_point, trace=False):
    x, wg_bf, bias_dup, sa, zp, recip, corr = _prep(
        x, weight, bias, scale_a, scale_w, zero_point)
    nc = _build(sa, -zp, recip)
    n = x.shape[0]
    assert n == N_CORES * IMGS, f"expected batch {N_CORES * IMGS}, got {n}"
    in_maps = [
        {"x": np.ascontiguousarray(x[k * IMGS:(k + 1) * IMGS]),
         "wt": wg_bf, "biasd": bias_dup}
        for k in range(N_CORES)
    ]
    try:
        res = bass_utils.run_bass_kernel_spmd(
            nc, in_maps, core_ids=list(range(N_CORES)), trace=trace)
    except ModuleNotFoundError:
        # axon NTFF profile hook unavailable in this environment
        res = bass_utils.run_bass_kernel_spmd(
            nc, in_maps, core_ids=list(range(N_CORES)), trace=False)
    y = np.concatenate([res.results[k]["y"] for k in range(N_CORES)], axis=0)
    if corr is not None:
        y = y + corr[None]
    return np.ascontiguousarray(y.astype(np.float32)), res


def kernel(x, weight, bias, scale_a, scale_w, zero_point):
    y, _ = _run(x, weight, bias, scale_a, scale_w, zero_point, trace=False)
    return y
